# revision 52
# baseline (speedup 1.0000x reference)
"""LIF neuron (no reset) Trainium2 kernel.

h_t = 0.5*h_{t-1} + 0.5*x_t ; spike_t = (h_t >= 1.0), x: [T=32, B=64, N=32768] f32.

Sharding: pure data-parallel over batch dim (dim 1) across 8 NeuronCores;
each core scans its [32, 8, 32768] shard over time. Per timestep the
262144-element slab is viewed as [128 partitions, 2048].

Recurrence (bit-exact vs the jax reference): the reference computes
h_t = fl(fl(0.5h)+fl(0.5x)) = fl(0.5(h+x)) (0.5-scales are exact).  With
S_t = 2^{t+1} h_t the recurrence is S_t = fl(S_{t-1} + 2^t x_t) (power-of-2
scaling commutes with round-to-nearest) and spike = (S_t >= 2^{t+1}) exactly.
One DVE op per step instead of two; |S| <= ~2^35, far from fp32 limits.

Output traffic is the kernel's lever: spikes are 0/1, so the device stores
them as 8-bit instead of f32 (8 MiB instead of 32 MiB per core on top of the
mandatory 32 MiB input read), and the host unshard step widens to f32.

Variants:
  sign8   - recurrence on DVE, spike via ACT Sign(1 - S*2^-(t+1)) -> int8.
            Decode: spike = (y < 0). S == threshold exactly gives 0 instead
            of 1 (P ~ 1e-8/elem; well inside the 2e-2 rel-err budget).
  mix8    - sign8 with the spike op split DVE/ACT per column half (no gain).
  isge8   - recurrence on DVE, spike via DVE is_ge -> uint8 (true >=).
  fused_f32 - recurrence + is_ge on DVE, f32 spikes (32 MiB stores).
  full_f32  - previous baseline: 3 DVE ops/step, f32 spikes.

Measured (reps-slope, 8 cores concurrent): full_f32 baseline 215.6 us;
sign8 default config ~127-129 us/rep.  The wall is the per-NC HBM *read*
path (~285 GB/s regardless of queue count: sync/scalar/gpsimd splits and
partition-major DRAM layouts were all tried).  Within one queue the SDMA
waits for each DMA's last-byte receipt before starting the next, so the
store packets on the other HWDGE ring are what fill those gaps: with
fine-grained stores (ss=2) the loads+stores floor beats pure loads, and
fine-grained deeply-prefetched loads (ls=2, 10 bufs — the SBUF limit)
make the compute coupling free, beating 8-step loads by ~5 us.  h is
ping-ponged across two buffers so the ACT Sign of step t overlaps the
DVE recurrence of step t+1 instead of WAR-serializing the engines (that
serialization cost ~80 us in the computeonly diagnostic).
"""

import numpy as np

import concourse.mybir as mybir
import concourse.tile as tile
from concourse import bacc
from concourse.bass_utils import run_bass_kernel_spmd

T, B, N = 32, 64, 32768
NCORES = 8
B_SH = B // NCORES            # 8 batch rows per core
E = B_SH * N                  # 262144 elements per timestep per core
P = 128                       # SBUF partitions

VARIANT = "sign8"             # kernel() default
LOAD_STEPS = 2                # timesteps per input DMA (2 MiB)
STORE_STEPS = 2               # timesteps per 8-bit output DMA (512 KiB)
XC_BUFS = 10
SP_BUFS = 6
DRAM_LAYOUT = "te"            # plain [T, E] device layout ("pt" measured no
                              # faster: the ~285 GB/s read path, not the 8 KiB
                              # descriptor size, is the limiter)

_prog_cache: dict = {}

_YDT = {
    "sign8": mybir.dt.int8,
    "g8": mybir.dt.int8,
    "mix8": mybir.dt.int8,
    "isge8": mybir.dt.uint8,
    "fused_f32": mybir.dt.float32,
    "full_f32": mybir.dt.float32,
    # diagnostics (bench_io only; y contents are meaningless)
    "loadonly": mybir.dt.float32,
    "storeonly": mybir.dt.int8,
    "dmaonly": mybir.dt.int8,
    "computeonly": mybir.dt.int8,
    "dveonly": mybir.dt.int8,
    "nosign8": mybir.dt.float32,   # loads + recurrence, token stores
    "nostore8": mybir.dt.int8,     # loads + recurrence + Sign, token stores
}
_DIAG = ("loadonly", "storeonly", "dmaonly", "computeonly", "dveonly")


def build_program(reps: int = 1, t_steps: int = T, e: int = E,
                  variant: str = VARIANT,
                  load_steps: int = LOAD_STEPS,
                  store_steps: int = STORE_STEPS,
                  xc_bufs: int = XC_BUFS, sp_bufs: int = SP_BUFS,
                  alt_rings: bool = False, store_ring: str = "scalar",
                  dram_layout: str = "te", unroll: int = 1,
                  h_depth: int = 2,
                  bench_io: bool = False):
    """Per-core Bass program: x[t_steps, e] f32 -> y[t_steps, e] spikes.

    reps>1 repeats the whole scan (h re-zeroed each rep) inside a hardware
    For_i loop for wall-clock HW timing: slope of t(reps) is pure
    per-iteration device time, cancelling RPC/compile/host-transfer
    overhead.  bench_io=True swaps the big external tensors for Internal
    DRAM scratch (tiny external in/out), so timed calls move ~KB over the
    host link instead of ~256 MiB; the DMA work per rep is unchanged.
    """
    f = e // P
    assert e % P == 0 and t_steps % load_steps == 0
    ydt = _YDT[variant]
    if variant in ("fused_f32", "full_f32"):
        store_steps = load_steps   # spikes overwrite xc in place

    # dram_layout="pt": host pre-packs x so each partition's load chunk is
    # one contiguous 64 KiB DRAM run (128 big descriptors per DMA instead of
    # 1024 8-KiB ones); y is packed the same way and unpacked on the host.
    packed = dram_layout == "pt"
    xshape = ([t_steps // load_steps, P * load_steps * f] if packed
              else [t_steps, e])
    yshape = ([t_steps // store_steps, P * store_steps * f] if packed
              else [t_steps, e])

    nc = bacc.Bacc()
    if bench_io:
        x_ext = nc.declare_dram_parameter("x", [P, 4], mybir.dt.float32,
                                          isOutput=False)
        y_ext = nc.declare_dram_parameter("y", [P, 4], mybir.dt.float32,
                                          isOutput=True)
        x = nc.dram_tensor("xs", xshape, mybir.dt.float32, kind="Internal")
        y = nc.dram_tensor("ys", yshape, ydt, kind="Internal")
    else:
        x = nc.declare_dram_parameter("x", xshape, mybir.dt.float32,
                                      isOutput=False)
        y = nc.declare_dram_parameter("y", yshape, ydt, isOutput=True)

    def load_ap(t0):
        if packed:
            return x[t0 // load_steps, :].rearrange(
                "(p t f) -> p t f", p=P, t=load_steps)
        return x[t0:t0 + load_steps, :].rearrange("t (p f) -> p t f", p=P)

    def store_ap(t0):
        if packed:
            return y[t0 // store_steps, :].rearrange(
                "(p t f) -> p t f", p=P, t=store_steps)
        return y[t0:t0 + store_steps, :].rearrange("t (p f) -> p t f", p=P)

    with tile.TileContext(nc) as tc:
        with (
            tc.tile_pool(name="xp", bufs=xc_bufs) as xp,
            tc.tile_pool(name="spp", bufs=sp_bufs) as spp,
            tc.tile_pool(name="tp", bufs=2) as tp,
            tc.tile_pool(name="hp", bufs=1) as hp,
        ):
            h = hp.tile([P, f], mybir.dt.float32, name="h")

            if bench_io:
                # fill the x scratch once (outside the timed loop) so the
                # streamed data is ordinary finite floats, and touch the tiny
                # external input so it isn't dead
                xe = hp.tile([P, 4], mybir.dt.float32, name="xe")
                nc.sync.dma_start(xe[:], x_ext[:, :])
                nc.vector.memset(h[:], 0.25)
                nc.vector.tensor_add(h[:, :4], h[:, :4], xe[:])
                for t in range(t_steps):
                    nc.sync.dma_start(
                        load_ap((t // load_steps) * load_steps)
                        [:, t % load_steps, :], h[:])

            if variant in _DIAG:
                assert bench_io
                if variant in ("storeonly", "dmaonly"):
                    spc = hp.tile([P, store_steps, f], mybir.dt.int8,
                                  name="spc")
                    nc.vector.memset(spc[:], 0)
                if variant in ("computeonly", "dveonly"):
                    xr = hp.tile([P, load_steps, f], mybir.dt.float32,
                                 name="xr")
                    nc.vector.memset(xr[:], 0.25)

                def diag_body(_i=None):
                    if variant in ("computeonly", "dveonly"):
                        nc.vector.memset(h[:], 0.0)
                        for t in range(t_steps):
                            nc.vector.scalar_tensor_tensor(
                                h[:], xr[:, t % load_steps, :],
                                float(2.0 ** t), h[:],
                                mybir.AluOpType.mult, mybir.AluOpType.add)
                            if variant == "computeonly":
                                if t % store_steps == 0:
                                    sp = spp.tile([P, store_steps, f],
                                                  mybir.dt.int8, name="sp",
                                                  tag="sp")
                                nc.scalar.activation(
                                    sp[:, t % store_steps, :], h[:],
                                    mybir.ActivationFunctionType.Sign,
                                    bias=1.0,
                                    scale=float(-(2.0 ** -(t + 1))))
                        return
                    if variant in ("loadonly", "dmaonly"):
                        for c in range(t_steps // load_steps):
                            t0 = c * load_steps
                            xc = xp.tile([P, load_steps, f],
                                         mybir.dt.float32, name="xc",
                                         tag="xc")
                            if alt_rings == "tri":
                                eng = [nc.sync, nc.scalar, nc.gpsimd][c % 3]
                            elif alt_rings and c % 2 == 1:
                                eng = getattr(
                                    nc, alt_rings if isinstance(alt_rings, str)
                                    else "scalar")
                            else:
                                eng = nc.sync
                            eng.dma_start(xc[:], load_ap(t0))
                            if variant == "loadonly":
                                # token store keeps xc alive, ~no traffic
                                yrow = min(t0, yshape[0] - 1)
                                nc.scalar.dma_start(
                                    y[yrow, :P * 2].rearrange(
                                        "(p f) -> p f", p=P),
                                    xc[:, 0, :2])
                    if variant in ("storeonly", "dmaonly"):
                        for s in range(t_steps // store_steps):
                            nc.scalar.dma_start(
                                store_ap(s * store_steps), spc[:])
                body = diag_body
            else:
                body = None

            # rotating state: step t writes hh[(t+1)%d] reading hh[t%d], so
            # the ACT Sign of step t overlaps the DVE recurrence of step t+1
            # instead of WAR-serializing the two engines on a single h
            # buffer; depth 3 tolerates one extra step of ACT lag
            hh = [h] + [hp.tile([P, f], mybir.dt.float32, name=f"hb{k}")
                        for k in range(h_depth - 1)]
            if variant == "g8":
                # constant-threshold state G_t = 0.5*G_{t-1} + x_t = 2*h_t
                # (bit-exact: the halving is a power-of-2 scale).  Steps land
                # in columns of two rotating [P, W, f] tiles, so one ACT Sign
                # covers W steps (same scale/bias for every step now) and its
                # output tile is the store tile.
                gw = store_steps
                gg = [hp.tile([P, gw, f], mybir.dt.float32, name=f"gg{k}")
                      for k in range(2)]

            def main_body(_i=None):
                if variant == "full_f32":
                    nc.vector.memset(h[:], 0.0)
                xc = None
                sp = None
                for t in range(t_steps):
                    if t % load_steps == 0:
                        xc = xp.tile([P, load_steps, f], mybir.dt.float32,
                                     name="xc", tag="xc")
                        # loads ride the SP HWDGE ring, stores the ACT ring,
                        # so the two directions overlap instead of
                        # serializing on one descriptor FIFO
                        ldeng = nc.sync
                        if alt_rings and (t // load_steps) % 2 == 1:
                            ldeng = getattr(
                                nc, alt_rings if isinstance(alt_rings, str)
                                else "scalar")
                        ldeng.dma_start(xc[:], load_ap(t))
                    dt = t % load_steps

                    if variant == "full_f32":
                        tmp = tp.tile([P, f], mybir.dt.float32,
                                      name="tmp", tag="tmp")
                        nc.vector.tensor_add(tmp[:], h[:], xc[:, dt, :])
                        nc.vector.tensor_scalar_mul(h[:], tmp[:], 0.5)
                        nc.vector.tensor_scalar(
                            xc[:, dt, :], tmp[:], 2.0, None,
                            mybir.AluOpType.is_ge)
                    elif variant == "g8":
                        gw = store_steps
                        cur = gg[(t // gw) % 2]
                        col = t % gw
                        if t == 0:
                            nc.vector.tensor_scalar_mul(
                                cur[:, 0, :], xc[:, dt, :], 1.0)
                        else:
                            prev = gg[((t - 1) // gw) % 2]
                            nc.vector.scalar_tensor_tensor(
                                cur[:, col, :], prev[:, (t - 1) % gw, :],
                                0.5, xc[:, dt, :],
                                mybir.AluOpType.mult, mybir.AluOpType.add)
                        if col == gw - 1:
                            sp = spp.tile([P, gw, f], ydt, name="sp",
                                          tag="sp")
                            # sign(1 - 0.5*G): spike = (y < 0) <=> G > 2
                            nc.scalar.activation(
                                sp[:], cur[:],
                                mybir.ActivationFunctionType.Sign,
                                bias=1.0, scale=-0.5)
                    else:
                        hprev = hh[t % h_depth]
                        hcur = hh[(t + 1) % h_depth]
                        if t == 0:
                            # S_0 = x_0: plain copy, no memset needed
                            nc.vector.tensor_scalar_mul(
                                hcur[:], xc[:, dt, :], 1.0)
                        else:
                            nc.vector.scalar_tensor_tensor(
                                hcur[:], xc[:, dt, :], float(2.0 ** t),
                                hprev[:],
                                mybir.AluOpType.mult, mybir.AluOpType.add)
                        if variant == "fused_f32":
                            nc.vector.tensor_scalar(
                                xc[:, dt, :], hcur[:], float(2.0 ** (t + 1)),
                                None, mybir.AluOpType.is_ge)
                        elif variant == "nosign8":
                            pass
                        else:
                            if t % store_steps == 0:
                                sp = spp.tile([P, store_steps, f], ydt,
                                              name="sp", tag="sp")
                            st = t % store_steps
                            if variant == "isge8":
                                nc.vector.tensor_scalar(
                                    sp[:, st, :], hcur[:],
                                    float(2.0 ** (t + 1)), None,
                                    mybir.AluOpType.is_ge)
                            elif variant == "mix8":
                                # DVE half: -(S>=thr) in {0,-1}; ACT half:
                                # sign(1-S*2^-(t+1)) in {-1,0,1}. Both mark a
                                # spike as negative, one uniform host decode.
                                fh = f // 2
                                nc.vector.tensor_scalar(
                                    sp[:, st, :fh], hcur[:, :fh],
                                    float(2.0 ** (t + 1)), -1.0,
                                    mybir.AluOpType.is_ge,
                                    mybir.AluOpType.mult)
                                nc.scalar.activation(
                                    sp[:, st, fh:], hcur[:, fh:],
                                    mybir.ActivationFunctionType.Sign,
                                    bias=1.0,
                                    scale=float(-(2.0 ** -(t + 1))))
                            else:  # sign8: sign(1 - S*2^-(t+1)); spike = (y<0)
                                nc.scalar.activation(
                                    sp[:, st, :], hcur[:],
                                    mybir.ActivationFunctionType.Sign,
                                    bias=1.0,
                                    scale=float(-(2.0 ** -(t + 1))))

                    if variant in ("fused_f32", "full_f32"):
                        if (t + 1) % load_steps == 0:
                            nc.scalar.dma_start(
                                store_ap(t + 1 - load_steps), xc[:])
                    elif (t + 1) % store_steps == 0:
                        if variant == "nosign8":    # token store from state
                            nc.scalar.dma_start(
                                y[0, :P * 2].rearrange("(p f) -> p f", p=P),
                                hcur[:, :2])
                        elif variant == "nostore8":  # token store from sp
                            nc.scalar.dma_start(
                                y[0, :P * 2].rearrange("(p f) -> p f", p=P),
                                sp[:, 0, :2])
                        else:
                            getattr(nc, store_ring).dma_start(
                                store_ap(t + 1 - store_steps), sp[:])

            if body is None:
                body = main_body
            if reps == 1:
                body()
            else:
                assert reps % unroll == 0
                with tc.For_i(0, reps // unroll, 1) as i:
                    for _ in range(unroll):
                        body(i)

            if bench_io:
                nc.scalar.dma_start(y_ext[:, :], h[:, :4])
    nc.compile()
    return nc


def _pack_x(x_core: np.ndarray, ls: int = LOAD_STEPS) -> np.ndarray:
    """[T, E] f32 -> [T//ls, P*ls*f]: each partition's chunk contiguous."""
    fdim = E // P
    return np.ascontiguousarray(
        x_core.reshape(T // ls, ls, P, fdim).transpose(0, 2, 1, 3)
    ).reshape(T // ls, P * ls * fdim)


def _decode(yshards: list[np.ndarray], variant: str,
            layout: str = DRAM_LAYOUT, ss: int = STORE_STEPS) -> np.ndarray:
    fdim = E // P
    out = np.empty((T, B, N), dtype=np.float32)
    for i, r in enumerate(yshards):
        if layout == "pt":
            r = r.reshape(T // ss, P, ss, fdim).transpose(0, 2, 1, 3)
        r = r.reshape(T, B_SH, N)
        dst = out[:, i * B_SH:(i + 1) * B_SH, :]
        if r.dtype == np.int8:        # sign8: {-1,0,1} -> spike = (y < 0)
            np.less(r, 0, out=dst, casting="unsafe")
        else:                         # isge8 uint8 {0,1} / f32 variants
            dst[...] = r
    return out


def run_sharded(x: np.ndarray, nc, variant: str = VARIANT,
                layout: str = DRAM_LAYOUT) -> np.ndarray:
    """Shard [T,B,N] over batch across 8 cores, run, gather."""
    shards = [
        np.ascontiguousarray(x[:, i * B_SH:(i + 1) * B_SH, :]).reshape(T, E)
        for i in range(NCORES)
    ]
    if layout == "pt":
        shards = [_pack_x(s) for s in shards]
    in_maps = [{"x": s} for s in shards]
    res = run_bass_kernel_spmd(nc, in_maps, list(range(NCORES)))
    return _decode([r["y"] for r in res.results], variant, layout)


def kernel(x_seq: np.ndarray) -> np.ndarray:
    x = np.asarray(x_seq, dtype=np.float32)
    assert x.shape == (T, B, N), x.shape
    if VARIANT not in _prog_cache:
        _prog_cache[VARIANT] = build_program(variant=VARIANT,
                                             dram_layout=DRAM_LAYOUT)
    return run_sharded(x, _prog_cache[VARIANT], VARIANT, DRAM_LAYOUT)


# revision 53
# speedup vs baseline: 1.0376x; 1.0376x over previous
"""LIF neuron (no reset) Trainium2 kernel.

h_t = 0.5*h_{t-1} + 0.5*x_t ; spike_t = (h_t >= 1.0), x: [T=32, B=64, N=32768] f32.

Sharding: pure data-parallel over batch dim (dim 1) across 8 NeuronCores;
each core scans its [32, 8, 32768] shard over time. Per timestep the
262144-element slab is viewed as [128 partitions, 2048].

Recurrence (bit-exact vs the jax reference): the reference computes
h_t = fl(fl(0.5h)+fl(0.5x)) = fl(0.5(h+x)) (0.5-scales are exact).  With
S_t = 2^{t+1} h_t the recurrence is S_t = fl(S_{t-1} + 2^t x_t) (power-of-2
scaling commutes with round-to-nearest) and spike = (S_t >= 2^{t+1}) exactly.
One DVE op per step instead of two; |S| <= ~2^35, far from fp32 limits.

Output traffic is the kernel's lever: spikes are 0/1, so the device stores
them as 8-bit instead of f32 (8 MiB instead of 32 MiB per core on top of the
mandatory 32 MiB input read), and the host unshard step widens to f32.

Variants:
  sign8   - recurrence on DVE, spike via ACT Sign(1 - S*2^-(t+1)) -> int8.
            Decode: spike = (y < 0). S == threshold exactly gives 0 instead
            of 1 (P ~ 1e-8/elem; well inside the 2e-2 rel-err budget).
  mix8    - sign8 with the spike op split DVE/ACT per column half (no gain).
  isge8   - recurrence on DVE, spike via DVE is_ge -> uint8 (true >=).
  fused_f32 - recurrence + is_ge on DVE, f32 spikes (32 MiB stores).
  full_f32  - previous baseline: 3 DVE ops/step, f32 spikes.

Measured (reps-slope, 8 cores concurrent): full_f32 baseline 215.6 us;
sign8 default config ~127-131 us/rep (run-to-run device variance ~3 us).  The wall is the per-NC HBM *read*
path (~285 GB/s regardless of queue count: sync/scalar/gpsimd splits and
partition-major DRAM layouts were all tried).  Within one queue the SDMA
waits for each DMA's last-byte receipt before starting the next, so the
store packets on the other HWDGE ring are what fill those gaps: with
fine-grained stores (ss=2) the loads+stores floor beats pure loads, and
fine-grained deeply-prefetched loads (ls=2, 10 bufs — the SBUF limit)
make the compute coupling free, beating 8-step loads by ~5 us.  h is
ping-ponged across two buffers so the ACT Sign of step t overlaps the
DVE recurrence of step t+1 instead of WAR-serializing the engines (that
serialization cost ~80 us in the computeonly diagnostic).
"""

import numpy as np

import concourse.mybir as mybir
import concourse.tile as tile
from concourse import bacc
from concourse.bass_utils import run_bass_kernel_spmd

T, B, N = 32, 64, 32768
NCORES = 8
B_SH = B // NCORES            # 8 batch rows per core
E = B_SH * N                  # 262144 elements per timestep per core
P = 128                       # SBUF partitions

VARIANT = "sign8"             # kernel() default
LOAD_STEPS = 2                # timesteps per input DMA (2 MiB)
STORE_STEPS = 2               # timesteps per 8-bit output DMA (512 KiB)
XC_BUFS = 10
SP_BUFS = 6
DRAM_LAYOUT = "te"            # plain [T, E] device layout ("pt" measured no
                              # faster: the ~285 GB/s read path, not the 8 KiB
                              # descriptor size, is the limiter)

_prog_cache: dict = {}

_YDT = {
    "sign8": mybir.dt.int8,
    "g8": mybir.dt.int8,
    "mix8": mybir.dt.int8,
    "isge8": mybir.dt.uint8,
    "fused_f32": mybir.dt.float32,
    "full_f32": mybir.dt.float32,
    # diagnostics (bench_io only; y contents are meaningless)
    "loadonly": mybir.dt.float32,
    "storeonly": mybir.dt.int8,
    "dmaonly": mybir.dt.int8,
    "computeonly": mybir.dt.int8,
    "dveonly": mybir.dt.int8,
    "nosign8": mybir.dt.float32,   # loads + recurrence, token stores
    "nostore8": mybir.dt.int8,     # loads + recurrence + Sign, token stores
}
_DIAG = ("loadonly", "storeonly", "dmaonly", "computeonly", "dveonly")


def build_program(reps: int = 1, t_steps: int = T, e: int = E,
                  variant: str = VARIANT,
                  load_steps: int = LOAD_STEPS,
                  store_steps: int = STORE_STEPS,
                  xc_bufs: int = XC_BUFS, sp_bufs: int = SP_BUFS,
                  alt_rings: bool = False, store_ring: str = "scalar",
                  dram_layout: str = "te", unroll: int = 1,
                  h_depth: int = 2,
                  bench_io: bool = False):
    """Per-core Bass program: x[t_steps, e] f32 -> y[t_steps, e] spikes.

    reps>1 repeats the whole scan (h re-zeroed each rep) inside a hardware
    For_i loop for wall-clock HW timing: slope of t(reps) is pure
    per-iteration device time, cancelling RPC/compile/host-transfer
    overhead.  bench_io=True swaps the big external tensors for Internal
    DRAM scratch (tiny external in/out), so timed calls move ~KB over the
    host link instead of ~256 MiB; the DMA work per rep is unchanged.
    """
    f = e // P
    assert e % P == 0 and t_steps % load_steps == 0
    ydt = _YDT[variant]
    if variant in ("fused_f32", "full_f32"):
        store_steps = load_steps   # spikes overwrite xc in place

    # dram_layout="pt": host pre-packs x so each partition's load chunk is
    # one contiguous 64 KiB DRAM run (128 big descriptors per DMA instead of
    # 1024 8-KiB ones); y is packed the same way and unpacked on the host.
    packed = dram_layout == "pt"
    xshape = ([t_steps // load_steps, P * load_steps * f] if packed
              else [t_steps, e])
    yshape = ([t_steps // store_steps, P * store_steps * f] if packed
              else [t_steps, e])

    nc = bacc.Bacc()
    if bench_io:
        x_ext = nc.declare_dram_parameter("x", [P, 4], mybir.dt.float32,
                                          isOutput=False)
        y_ext = nc.declare_dram_parameter("y", [P, 4], mybir.dt.float32,
                                          isOutput=True)
        x = nc.dram_tensor("xs", xshape, mybir.dt.float32, kind="Internal")
        y = nc.dram_tensor("ys", yshape, ydt, kind="Internal")
    else:
        x = nc.declare_dram_parameter("x", xshape, mybir.dt.float32,
                                      isOutput=False)
        y = nc.declare_dram_parameter("y", yshape, ydt, isOutput=True)

    def load_ap(t0):
        if packed:
            return x[t0 // load_steps, :].rearrange(
                "(p t f) -> p t f", p=P, t=load_steps)
        return x[t0:t0 + load_steps, :].rearrange("t (p f) -> p t f", p=P)

    def store_ap(t0):
        if packed:
            return y[t0 // store_steps, :].rearrange(
                "(p t f) -> p t f", p=P, t=store_steps)
        return y[t0:t0 + store_steps, :].rearrange("t (p f) -> p t f", p=P)

    with tile.TileContext(nc) as tc:
        with (
            tc.tile_pool(name="xp", bufs=xc_bufs) as xp,
            tc.tile_pool(name="spp", bufs=sp_bufs) as spp,
            tc.tile_pool(name="tp", bufs=2) as tp,
            tc.tile_pool(name="hp", bufs=1) as hp,
        ):
            h = hp.tile([P, f], mybir.dt.float32, name="h")

            if bench_io:
                # fill the x scratch once (outside the timed loop) so the
                # streamed data is ordinary finite floats, and touch the tiny
                # external input so it isn't dead
                xe = hp.tile([P, 4], mybir.dt.float32, name="xe")
                nc.sync.dma_start(xe[:], x_ext[:, :])
                nc.vector.memset(h[:], 0.25)
                nc.vector.tensor_add(h[:, :4], h[:, :4], xe[:])
                for t in range(t_steps):
                    nc.sync.dma_start(
                        load_ap((t // load_steps) * load_steps)
                        [:, t % load_steps, :], h[:])

            if variant in _DIAG:
                assert bench_io
                if variant in ("storeonly", "dmaonly"):
                    spc = hp.tile([P, store_steps, f], mybir.dt.int8,
                                  name="spc")
                    nc.vector.memset(spc[:], 0)
                if variant in ("computeonly", "dveonly"):
                    xr = hp.tile([P, load_steps, f], mybir.dt.float32,
                                 name="xr")
                    nc.vector.memset(xr[:], 0.25)

                def diag_body(_i=None):
                    if variant in ("computeonly", "dveonly"):
                        nc.vector.memset(h[:], 0.0)
                        for t in range(t_steps):
                            nc.vector.scalar_tensor_tensor(
                                h[:], xr[:, t % load_steps, :],
                                float(2.0 ** t), h[:],
                                mybir.AluOpType.mult, mybir.AluOpType.add)
                            if variant == "computeonly":
                                if t % store_steps == 0:
                                    sp = spp.tile([P, store_steps, f],
                                                  mybir.dt.int8, name="sp",
                                                  tag="sp")
                                nc.scalar.activation(
                                    sp[:, t % store_steps, :], h[:],
                                    mybir.ActivationFunctionType.Sign,
                                    bias=1.0,
                                    scale=float(-(2.0 ** -(t + 1))))
                        return
                    if variant in ("loadonly", "dmaonly"):
                        for c in range(t_steps // load_steps):
                            t0 = c * load_steps
                            xc = xp.tile([P, load_steps, f],
                                         mybir.dt.float32, name="xc",
                                         tag="xc")
                            if alt_rings == "tri":
                                eng = [nc.sync, nc.scalar, nc.gpsimd][c % 3]
                            elif alt_rings and c % 2 == 1:
                                eng = getattr(
                                    nc, alt_rings if isinstance(alt_rings, str)
                                    else "scalar")
                            else:
                                eng = nc.sync
                            eng.dma_start(xc[:], load_ap(t0))
                            if variant == "loadonly":
                                # token store keeps xc alive, ~no traffic
                                yrow = min(t0, yshape[0] - 1)
                                nc.scalar.dma_start(
                                    y[yrow, :P * 2].rearrange(
                                        "(p f) -> p f", p=P),
                                    xc[:, 0, :2])
                    if variant in ("storeonly", "dmaonly"):
                        for s in range(t_steps // store_steps):
                            nc.scalar.dma_start(
                                store_ap(s * store_steps), spc[:])
                body = diag_body
            else:
                body = None

            # rotating state: step t writes hh[(t+1)%d] reading hh[t%d], so
            # the ACT Sign of step t overlaps the DVE recurrence of step t+1
            # instead of WAR-serializing the two engines on a single h
            # buffer; depth 3 tolerates one extra step of ACT lag
            hh = [h] + [hp.tile([P, f], mybir.dt.float32, name=f"hb{k}")
                        for k in range(h_depth - 1)]
            if variant == "g8":
                # constant-threshold state G_t = 0.5*G_{t-1} + x_t = 2*h_t
                # (bit-exact: the halving is a power-of-2 scale).  Steps land
                # in columns of two rotating [P, W, f] tiles, so one ACT Sign
                # covers W steps (same scale/bias for every step now) and its
                # output tile is the store tile.
                gw = store_steps
                gg = [hp.tile([P, gw, f], mybir.dt.float32, name=f"gg{k}")
                      for k in range(2)]

            def main_body(_i=None):
                if variant == "full_f32":
                    nc.vector.memset(h[:], 0.0)
                xc = None
                sp = None
                for t in range(t_steps):
                    if t % load_steps == 0:
                        xc = xp.tile([P, load_steps, f], mybir.dt.float32,
                                     name="xc", tag="xc")
                        # loads ride the SP HWDGE ring, stores the ACT ring,
                        # so the two directions overlap instead of
                        # serializing on one descriptor FIFO
                        ldeng = nc.sync
                        if alt_rings and (t // load_steps) % 2 == 1:
                            ldeng = getattr(
                                nc, alt_rings if isinstance(alt_rings, str)
                                else "scalar")
                        ldeng.dma_start(xc[:], load_ap(t))
                    dt = t % load_steps

                    if variant == "full_f32":
                        tmp = tp.tile([P, f], mybir.dt.float32,
                                      name="tmp", tag="tmp")
                        nc.vector.tensor_add(tmp[:], h[:], xc[:, dt, :])
                        nc.vector.tensor_scalar_mul(h[:], tmp[:], 0.5)
                        nc.vector.tensor_scalar(
                            xc[:, dt, :], tmp[:], 2.0, None,
                            mybir.AluOpType.is_ge)
                    elif variant == "g8":
                        gw = store_steps
                        cur = gg[(t // gw) % 2]
                        col = t % gw
                        if t == 0:
                            nc.vector.tensor_scalar_mul(
                                cur[:, 0, :], xc[:, dt, :], 1.0)
                        else:
                            prev = gg[((t - 1) // gw) % 2]
                            nc.vector.scalar_tensor_tensor(
                                cur[:, col, :], prev[:, (t - 1) % gw, :],
                                0.5, xc[:, dt, :],
                                mybir.AluOpType.mult, mybir.AluOpType.add)
                        if col == gw - 1:
                            sp = spp.tile([P, gw, f], ydt, name="sp",
                                          tag="sp")
                            # sign(1 - 0.5*G): spike = (y < 0) <=> G > 2
                            nc.scalar.activation(
                                sp[:], cur[:],
                                mybir.ActivationFunctionType.Sign,
                                bias=1.0, scale=-0.5)
                    else:
                        hprev = hh[t % h_depth]
                        hcur = hh[(t + 1) % h_depth]
                        if t == 0:
                            # S_0 = x_0: plain copy, no memset needed
                            nc.vector.tensor_scalar_mul(
                                hcur[:], xc[:, dt, :], 1.0)
                        else:
                            nc.vector.scalar_tensor_tensor(
                                hcur[:], xc[:, dt, :], float(2.0 ** t),
                                hprev[:],
                                mybir.AluOpType.mult, mybir.AluOpType.add)
                        if variant == "fused_f32":
                            nc.vector.tensor_scalar(
                                xc[:, dt, :], hcur[:], float(2.0 ** (t + 1)),
                                None, mybir.AluOpType.is_ge)
                        elif variant == "nosign8":
                            pass
                        else:
                            if t % store_steps == 0:
                                sp = spp.tile([P, store_steps, f], ydt,
                                              name="sp", tag="sp")
                            st = t % store_steps
                            if variant == "isge8":
                                nc.vector.tensor_scalar(
                                    sp[:, st, :], hcur[:],
                                    float(2.0 ** (t + 1)), None,
                                    mybir.AluOpType.is_ge)
                            elif variant == "mix8":
                                # DVE half: -(S>=thr) in {0,-1}; ACT half:
                                # sign(1-S*2^-(t+1)) in {-1,0,1}. Both mark a
                                # spike as negative, one uniform host decode.
                                fh = f // 2
                                nc.vector.tensor_scalar(
                                    sp[:, st, :fh], hcur[:, :fh],
                                    float(2.0 ** (t + 1)), -1.0,
                                    mybir.AluOpType.is_ge,
                                    mybir.AluOpType.mult)
                                nc.scalar.activation(
                                    sp[:, st, fh:], hcur[:, fh:],
                                    mybir.ActivationFunctionType.Sign,
                                    bias=1.0,
                                    scale=float(-(2.0 ** -(t + 1))))
                            else:  # sign8: sign(1 - S*2^-(t+1)); spike = (y<0)
                                nc.scalar.activation(
                                    sp[:, st, :], hcur[:],
                                    mybir.ActivationFunctionType.Sign,
                                    bias=1.0,
                                    scale=float(-(2.0 ** -(t + 1))))

                    if variant in ("fused_f32", "full_f32"):
                        if (t + 1) % load_steps == 0:
                            nc.scalar.dma_start(
                                store_ap(t + 1 - load_steps), xc[:])
                    elif (t + 1) % store_steps == 0:
                        if variant == "nosign8":    # token store from state
                            nc.scalar.dma_start(
                                y[0, :P * 2].rearrange("(p f) -> p f", p=P),
                                hcur[:, :2])
                        elif variant == "nostore8":  # token store from sp
                            nc.scalar.dma_start(
                                y[0, :P * 2].rearrange("(p f) -> p f", p=P),
                                sp[:, 0, :2])
                        else:
                            getattr(nc, store_ring).dma_start(
                                store_ap(t + 1 - store_steps), sp[:])

            if body is None:
                body = main_body
            if reps == 1:
                body()
            else:
                assert reps % unroll == 0
                with tc.For_i(0, reps // unroll, 1) as i:
                    for _ in range(unroll):
                        body(i)

            if bench_io:
                nc.scalar.dma_start(y_ext[:, :], h[:, :4])
    nc.compile()
    return nc


def _pack_x(x_core: np.ndarray, ls: int = LOAD_STEPS) -> np.ndarray:
    """[T, E] f32 -> [T//ls, P*ls*f]: each partition's chunk contiguous."""
    fdim = E // P
    return np.ascontiguousarray(
        x_core.reshape(T // ls, ls, P, fdim).transpose(0, 2, 1, 3)
    ).reshape(T // ls, P * ls * fdim)


def _decode(yshards: list[np.ndarray], variant: str,
            layout: str = DRAM_LAYOUT, ss: int = STORE_STEPS) -> np.ndarray:
    fdim = E // P
    out = np.empty((T, B, N), dtype=np.float32)
    for i, r in enumerate(yshards):
        if layout == "pt":
            r = r.reshape(T // ss, P, ss, fdim).transpose(0, 2, 1, 3)
        r = r.reshape(T, B_SH, N)
        dst = out[:, i * B_SH:(i + 1) * B_SH, :]
        if r.dtype == np.int8:        # sign8: {-1,0,1} -> spike = (y < 0)
            np.less(r, 0, out=dst, casting="unsafe")
        else:                         # isge8 uint8 {0,1} / f32 variants
            dst[...] = r
    return out


def run_sharded(x: np.ndarray, nc, variant: str = VARIANT,
                layout: str = DRAM_LAYOUT) -> np.ndarray:
    """Shard [T,B,N] over batch across 8 cores, run, gather."""
    shards = [
        np.ascontiguousarray(x[:, i * B_SH:(i + 1) * B_SH, :]).reshape(T, E)
        for i in range(NCORES)
    ]
    if layout == "pt":
        shards = [_pack_x(s) for s in shards]
    in_maps = [{"x": s} for s in shards]
    res = run_bass_kernel_spmd(nc, in_maps, list(range(NCORES)))
    return _decode([r["y"] for r in res.results], variant, layout)


def kernel(x_seq: np.ndarray) -> np.ndarray:
    x = np.asarray(x_seq, dtype=np.float32)
    assert x.shape == (T, B, N), x.shape
    if VARIANT not in _prog_cache:
        _prog_cache[VARIANT] = build_program(variant=VARIANT,
                                             dram_layout=DRAM_LAYOUT)
    return run_sharded(x, _prog_cache[VARIANT], VARIANT, DRAM_LAYOUT)
